# revision 15
# baseline (speedup 1.0000x reference)
# Multi-head attention (B=4, S=1024, H=16, D=64) on 8 trn2 NeuronCores.
#
# Sharding: core c handles batch b=c//2 and heads [8*(c%2), 8*(c%2)+8).
# Each core computes Q/K/V projections for its 512 head-dims over its
# batch's full sequence, per-head attention (scores^T layout: [k, q] so
# softmax sums come from a ones-column packed into the ctx matmul), and
# a partial output projection; the host sums the two half-head partials
# per batch and view-transposes the [k, q] attention back to [q, k].
#
# All device compute is bf16 on the PE with f32 PSUM accumulation; the
# attention output is written bf16 and upcast on the host.

import os
import sys

for _p in ("/opt/trn_rl_repo", "/root/.axon_site/_ro/trn_rl_repo"):
    if os.path.isdir(_p) and _p not in sys.path:
        sys.path.append(_p)

import numpy as np
import ml_dtypes

B, S, H, D = 4, 1024, 16, 64
DM = 1024      # model dim
NH = 8         # heads per core
DH = NH * D    # 512 head-dims per core
P = 128
KT = DM // P   # 8 contraction / seq tiles
OT = DH // P   # 4 out-dim tiles per core
NCORES = 8
BF16NP = ml_dtypes.bfloat16

_CACHE = {}


def _build_module():
    import concourse.tile as tile
    from concourse import bacc, mybir

    f32 = mybir.dt.float32
    bf16 = mybir.dt.bfloat16
    Exp = mybir.ActivationFunctionType.Exp
    Ln = mybir.ActivationFunctionType.Ln

    nc = bacc.Bacc(
        "TRN2", target_bir_lowering=False, debug=False, num_devices=NCORES
    )

    qT = nc.dram_tensor("qT", [DM, S], bf16, kind="ExternalInput").ap()
    kT = nc.dram_tensor("kT", [DM, S], bf16, kind="ExternalInput").ap()
    vT = nc.dram_tensor("vT", [DM, S], bf16, kind="ExternalInput").ap()
    wqT = nc.dram_tensor("wqT", [DM, DH], bf16, kind="ExternalInput").ap()
    wkT = nc.dram_tensor("wkT", [DM, DH], bf16, kind="ExternalInput").ap()
    wvT = nc.dram_tensor("wvT", [DM, DH], bf16, kind="ExternalInput").ap()
    woT = nc.dram_tensor("woT", [DH, DM], bf16, kind="ExternalInput").ap()
    # outputs in partition-major layouts so DMA runs are 16KB contiguous:
    # attnT[h, p, t, q] with k = t*128+p ; outT[p, mt, q] with dm = mt*128+p
    attnT = nc.dram_tensor(
        "attnT", [NH, P, KT, S], bf16, kind="ExternalOutput"
    ).ap()
    outT = nc.dram_tensor("outT", [P, KT, S], f32, kind="ExternalOutput").ap()

    with tile.TileContext(nc) as tc:
        with (
            tc.tile_pool(name="singles", bufs=1) as singles,
            tc.tile_pool(name="exps", bufs=3) as exps,
            tc.tile_pool(name="small", bufs=2) as small,
            tc.tile_pool(name="outsb", bufs=6) as outsb,
            tc.tile_pool(name="ps", bufs=2, space="PSUM") as psum,
            tc.tile_pool(name="ctxp", bufs=2, space="PSUM") as ctxpsum,
        ):
            # ---- stage inputs in SBUF ----
            qT_sb = singles.tile([P, KT, S], bf16)
            kT_sb = singles.tile([P, KT, S], bf16)
            vT_sb = singles.tile([P, KT, S], bf16)
            wq_sb = singles.tile([P, KT, DH], bf16)
            wk_sb = singles.tile([P, KT, DH], bf16)
            wv_sb = singles.tile([P, KT, DH], bf16)
            wo_sb = singles.tile([P, OT, DM], bf16)
            # host supplies partition-major [(p t), cols] so each whole-
            # tensor DMA moves one contiguous 8-16KB run per partition;
            # ordered so the Q projection's operands land first
            loads = [
                (wq_sb, wqT), (qT_sb, qT),
                (wk_sb, wkT), (kT_sb, kT),
                (wv_sb, wvT), (vT_sb, vT),
            ]
            for sb_t, dram_t in loads:
                nc.sync.dma_start(
                    out=sb_t,
                    in_=dram_t.rearrange("(p t) c -> p t c", p=P),
                )
            nc.sync.dma_start(out=wo_sb, in_=woT.rearrange("(p t) d -> p t d", p=P))
            ones_sb = singles.tile([1, P], f32)
            nc.vector.memset(ones_sb, 1.0)


            # QpT/KpT: [part = head-dim within tile, ot, seq]; head h lives on
            # partitions 64*(h%2).. of tile h//2.
            qp_sb = singles.tile([P, OT, S], bf16)
            kp_sb = singles.tile([P, OT, S], bf16)
            # Vp: [part = seq within tile, seq-tile, head, d + ones column]
            vp_sb = singles.tile([P, KT, NH, D + 1], bf16)
            # normalized ctx^T stacked: [part = head-dim within tile, ct, q]
            ctxn_sb = singles.tile([P, OT, S], bf16)

            def emit_qk_proj(ot):
                # QpT/KpT columns for head pair (2*ot, 2*ot+1)
                for w_sb, x_sb, dst in ((wq_sb, qT_sb, qp_sb), (wk_sb, kT_sb, kp_sb)):
                    for nch in range(2):
                        pslice = psum.tile([P, 512], f32, tag="ps", name=f"pj{ot}{nch}")
                        for kt in range(KT):
                            nc.tensor.matmul(
                                pslice,
                                lhsT=w_sb[:, kt, ot * 128 : (ot + 1) * 128],
                                rhs=x_sb[:, kt, nch * 512 : (nch + 1) * 512],
                                start=(kt == 0),
                                stop=(kt == KT - 1),
                            )
                        nc.vector.tensor_copy(
                            out=dst[:, ot, nch * 512 : (nch + 1) * 512], in_=pslice
                        )

            def emit_v_proj():
                nc.vector.memset(vp_sb[:, :, :, D], 1.0)
                for mt in range(KT):
                    pslice = psum.tile([P, 512], f32, tag="ps", name=f"pv{mt}")
                    for kt in range(KT):
                        nc.tensor.matmul(
                            pslice,
                            lhsT=vT_sb[:, kt, mt * 128 : (mt + 1) * 128],
                            rhs=wv_sb[:, kt, :],
                            start=(kt == 0),
                            stop=(kt == KT - 1),
                        )
                    nc.vector.tensor_copy(
                        out=vp_sb[:, mt, :, 0:D],
                        in_=pslice.rearrange("p (h d) -> p h d", h=NH),
                    )

            # ---- attention pipeline with the projections streamed into the
            # early cycles: scores/exp of head c interleave with the ctx
            # matmuls of head c-1; remaining projections are emitted between
            # cycles so the PE chews on them while ScalarE runs exp ----
            expTs = {}
            ctxs = {}
            rbcs = {}

            def cycle(c):
                if c < NH:
                    expTs[c] = exps.tile([P, KT, S], bf16, tag="expT", name=f"expT{c}")
                    ctxs[c] = ctxpsum.tile([P, S], f32, tag="ctx", name=f"ctx{c}")
                for kt in range(KT):
                    if c < NH:
                        hp = 64 * (c % 2)
                        ot = c // 2
                        for nch in range(2):
                            sc = psum.tile(
                                [P, 512], f32, tag="ps", name=f"sc{c}{kt}{nch}"
                            )
                            nc.tensor.matmul(
                                sc,
                                lhsT=kp_sb[
                                    hp : hp + 64, ot, kt * 128 : (kt + 1) * 128
                                ],
                                rhs=qp_sb[hp : hp + 64, ot, nch * 512 : (nch + 1) * 512],
                                start=True,
                                stop=True,
                            )
                            # exp(scores/8); bf16 out feeds both the ctx
                            # matmul and, after normalization, the attn output
                            nc.scalar.activation(
                                out=expTs[c][:, kt, nch * 512 : (nch + 1) * 512],
                                in_=sc,
                                func=Exp,
                                scale=1.0 / 8.0,
                            )
                    if c >= 1:
                        t = c - 1
                        for nch in range(2):
                            nc.tensor.matmul(
                                ctxs[t][0 : D + 1, nch * 512 : (nch + 1) * 512],
                                lhsT=vp_sb[:, kt, t, :],
                                rhs=expTs[t][:, kt, nch * 512 : (nch + 1) * 512],
                                start=(kt == 0),
                                stop=(kt == KT - 1),
                            )
                if c >= 1:
                    # tail part 1 for head t = c-1: normalizer + ctxn (the
                    # out-projection gate); the attn normalization and DMA
                    # (part 2) are deferred to the next cycle so they don't
                    # sit on this critical path
                    t = c - 1
                    ctx = ctxs.pop(t)
                    # shift the sums row to base partition 0 (engines can
                    # retarget partition bases), then the fast custom-DVE
                    # reciprocal (only correct at base 0 on HW)
                    sums = small.tile([1, S], f32, tag="sums")
                    nc.vector.tensor_copy(out=sums, in_=ctx[D : D + 1, :])
                    recf = small.tile([1, S], f32, tag="recf")
                    nc.vector.reciprocal_approx_fast(out=recf, in_=sums)
                    # broadcast 1/s across partitions via a K=1 PE matmul
                    rbc = small.tile([P, S], bf16, tag="rbc", name=f"rbc{t}")
                    for nch in range(2):
                        bc = psum.tile([P, 512], f32, tag="ps", name=f"bc{t}{nch}")
                        nc.tensor.matmul(
                            bc,
                            lhsT=ones_sb,
                            rhs=recf[:, nch * 512 : (nch + 1) * 512],
                            start=True,
                            stop=True,
                        )
                        nc.vector.tensor_copy(
                            out=rbc[:, nch * 512 : (nch + 1) * 512], in_=bc
                        )
                    # ctxn first: it gates the output projection
                    base = 64 * (t % 2)
                    nc.vector.tensor_mul(
                        out=ctxn_sb[base : base + 64, t // 2, :],
                        in0=ctx[0:D, :],
                        in1=rbc[0:D, :],
                    )
                    rbcs[t] = rbc

            def tail2(t):
                # attn normalization (one 3D op) + store, off the ctxn path
                expT = expTs.pop(t)
                rbc = rbcs.pop(t)
                nc.vector.tensor_mul(
                    out=expT,
                    in0=expT,
                    in1=rbc.unsqueeze(1).to_broadcast((P, KT, S)),
                )
                nc.sync.dma_start(out=attnT[t], in_=expT)

            emit_qk_proj(0)
            cycle(0)               # scores/exp head 0
            emit_v_proj()
            emit_qk_proj(1)
            cycle(1)               # + ctx head 0
            emit_qk_proj(2)
            tail2(0)
            cycle(2)
            emit_qk_proj(3)
            for c in range(3, NH + 1):
                tail2(c - 2)
                cycle(c)
            tail2(NH - 1)

            # ---- output projection: outT = woT.T @ ctxn ----
            for mt in range(KT):
                for nch in range(2):
                    pslice = psum.tile([P, 512], f32, tag="ps", name=f"po{mt}{nch}")
                    for ct in range(OT):
                        nc.tensor.matmul(
                            pslice,
                            lhsT=wo_sb[:, ct, mt * 128 : (mt + 1) * 128],
                            rhs=ctxn_sb[:, ct, nch * 512 : (nch + 1) * 512],
                            start=(ct == 0),
                            stop=(ct == OT - 1),
                        )
                    ob = outsb.tile([P, 512], f32, tag="ob")
                    nc.scalar.copy(out=ob, in_=pslice)
                    nc.sync.dma_start(
                        out=outT[:, mt, nch * 512 : (nch + 1) * 512], in_=ob
                    )

    nc.compile()
    return nc


def _get_nc():
    if "nc" not in _CACHE:
        _CACHE["nc"] = _build_module()
    return _CACHE["nc"]


def _make_in_maps(query, key, value, Wq, Wk, Wv, Wo):
    query, key, value, Wq, Wk, Wv, Wo = (
        np.asarray(x, dtype=np.float32) for x in (query, key, value, Wq, Wk, Wv, Wo)
    )
    in_maps = []
    for c in range(NCORES):
        b, half = divmod(c, 2)
        hs = slice(half * DH, (half + 1) * DH)
        def pmajor(arr2d):
            # [(t p), c] -> [(p t), c] so each SBUF partition's data is one
            # contiguous DRAM run
            r, cdim = arr2d.shape
            t = r // P
            return (
                arr2d.reshape(t, P, cdim).swapaxes(0, 1).reshape(r, cdim)
            )

        in_maps.append(
            {
                "qT": pmajor(query[b].T.astype(BF16NP)),
                "kT": pmajor(key[b].T.astype(BF16NP)),
                "vT": pmajor(value[b].T.astype(BF16NP)),
                "wqT": pmajor(Wq[hs, :].T.astype(BF16NP)),
                "wkT": pmajor(Wk[hs, :].T.astype(BF16NP)),
                "wvT": pmajor(Wv[hs, :].T.astype(BF16NP)),
                "woT": pmajor(Wo[:, hs].T.astype(BF16NP)),
            }
        )
    return in_maps


def _assemble(results):
    attn = np.empty((B, H, S, S), np.float32)
    out = np.empty((B, S, DM), np.float32)
    for b in range(B):
        r0, r1 = results[2 * b], results[2 * b + 1]
        # attnT is [h, p, t, q] with k = t*128+p; reference wants [h, q, k]
        for half, r in ((0, r0), (1, r1)):
            a = np.asarray(r["attnT"])  # [8, 128, 8, 1024]
            a = a.transpose(0, 3, 2, 1).reshape(NH, S, S)  # [h, q, (t p)=k]
            attn[b, half * NH : (half + 1) * NH] = a.astype(np.float32)
        # outT is [p, mt, q] with dm = mt*128+p
        o = np.asarray(r0["outT"]) + np.asarray(r1["outT"])
        out[b] = o.transpose(2, 1, 0).reshape(S, DM)
    return out, attn


def run(trace=False, **inputs):
    from concourse import bass_utils

    nc = _get_nc()
    in_maps = _make_in_maps(**inputs)
    res = bass_utils.run_bass_kernel_spmd(
        nc, in_maps, core_ids=list(range(NCORES)), trace=trace
    )
    _CACHE["last_result"] = res
    out, attn = _assemble(res.results)
    return out, attn


def kernel(query, key, value, Wq, Wk, Wv, Wo):
    return run(
        query=query, key=key, value=value, Wq=Wq, Wk=Wk, Wv=Wv, Wo=Wo
    )


# revision 17
# speedup vs baseline: 1.1370x; 1.1370x over previous
# Multi-head attention (B=4, S=1024, H=16, D=64) on 8 trn2 NeuronCores.
#
# Sharding: core c handles batch b=c//2 and heads [8*(c%2), 8*(c%2)+8).
# Each core computes Q/K/V projections for its 512 head-dims over its
# batch's full sequence, per-head attention (scores^T layout: [k, q] so
# softmax sums come from a ones-column packed into the ctx matmul), and
# a partial output projection; the host sums the two half-head partials
# per batch and view-transposes the [k, q] attention back to [q, k].
#
# All device compute is bf16 on the PE with f32 PSUM accumulation; the
# attention output is written bf16 and upcast on the host.

import os
import sys

for _p in ("/opt/trn_rl_repo", "/root/.axon_site/_ro/trn_rl_repo"):
    if os.path.isdir(_p) and _p not in sys.path:
        sys.path.append(_p)

import numpy as np
import ml_dtypes

B, S, H, D = 4, 1024, 16, 64
DM = 1024      # model dim
NH = 8         # heads per core
DH = NH * D    # 512 head-dims per core
P = 128
KT = DM // P   # 8 contraction / seq tiles
OT = DH // P   # 4 out-dim tiles per core
NCORES = 8
BF16NP = ml_dtypes.bfloat16

_CACHE = {}


def _build_module():
    import concourse.tile as tile
    from concourse import bacc, mybir

    f32 = mybir.dt.float32
    bf16 = mybir.dt.bfloat16
    Exp = mybir.ActivationFunctionType.Exp
    Ln = mybir.ActivationFunctionType.Ln

    nc = bacc.Bacc(
        "TRN2", target_bir_lowering=False, debug=False, num_devices=NCORES
    )

    qT = nc.dram_tensor("qT", [DM, S], bf16, kind="ExternalInput").ap()
    kT = nc.dram_tensor("kT", [DM, S], bf16, kind="ExternalInput").ap()
    vT = nc.dram_tensor("vT", [DM, S], bf16, kind="ExternalInput").ap()
    wqT = nc.dram_tensor("wqT", [DM, DH], bf16, kind="ExternalInput").ap()
    wkT = nc.dram_tensor("wkT", [DM, DH], bf16, kind="ExternalInput").ap()
    wvT = nc.dram_tensor("wvT", [DM, DH], bf16, kind="ExternalInput").ap()
    woT = nc.dram_tensor("woT", [DH, DM], bf16, kind="ExternalInput").ap()
    # outputs in partition-major layouts so DMA runs are 16KB contiguous:
    # attnT[h, p, t, q] with k = t*128+p ; outT[p, mt, q] with dm = mt*128+p
    attnT = nc.dram_tensor(
        "attnT", [NH, P, KT, S], bf16, kind="ExternalOutput"
    ).ap()
    outT = nc.dram_tensor("outT", [P, KT, S], f32, kind="ExternalOutput").ap()

    with tile.TileContext(nc) as tc:
        with (
            tc.tile_pool(name="singles", bufs=1) as singles,
            tc.tile_pool(name="exps", bufs=3) as exps,
            tc.tile_pool(name="small", bufs=2) as small,
            tc.tile_pool(name="outsb", bufs=6) as outsb,
            tc.tile_pool(name="scp", bufs=2, space="PSUM") as scp,
            tc.tile_pool(name="ctxp", bufs=1, space="PSUM") as ctxp,
            tc.tile_pool(name="bcp", bufs=1, space="PSUM") as bcp,
            tc.tile_pool(name="ppp", bufs=1, space="PSUM") as ppp,
        ):
            # ---- stage inputs in SBUF ----
            qT_sb = singles.tile([P, KT, S], bf16)
            kT_sb = singles.tile([P, KT, S], bf16)
            vT_sb = singles.tile([P, KT, S], bf16)
            wq_sb = singles.tile([P, KT, DH], bf16)
            wk_sb = singles.tile([P, KT, DH], bf16)
            wv_sb = singles.tile([P, KT, DH], bf16)
            wo_sb = singles.tile([P, OT, DM], bf16)
            # host supplies partition-major [(p t), cols] so each whole-
            # tensor DMA moves one contiguous 8-16KB run per partition;
            # ordered so the Q projection's operands land first
            loads = [
                (wq_sb, wqT), (qT_sb, qT),
                (wk_sb, wkT), (kT_sb, kT),
                (wv_sb, wvT), (vT_sb, vT),
            ]
            for sb_t, dram_t in loads:
                nc.sync.dma_start(
                    out=sb_t,
                    in_=dram_t.rearrange("(p t) c -> p t c", p=P),
                )
            nc.sync.dma_start(out=wo_sb, in_=woT.rearrange("(p t) d -> p t d", p=P))
            ones_sb = singles.tile([1, P], f32)
            nc.vector.memset(ones_sb, 1.0)

            # QpT/KpT: [part = head-dim within tile, ot, seq]; head h lives on
            # partitions 64*(h%2).. of tile h//2.
            qp_sb = singles.tile([P, OT, S], bf16)
            kp_sb = singles.tile([P, OT, S], bf16)
            # Vp: [part = seq within tile, seq-tile, head, d + ones column]
            vp_sb = singles.tile([P, KT, NH, D + 1], bf16)
            # normalized ctx^T stacked: [part = head-dim within tile, ct, q]
            ctxn_sb = singles.tile([P, OT, S], bf16)

            # ---- projection helpers ----
            def qk_group(which, ot, nch, pool):
                # one [128, 512] PSUM group of the Q or K projection
                w_sb, x_sb, dst = (
                    (wq_sb, qT_sb, qp_sb) if which == "q" else (wk_sb, kT_sb, kp_sb)
                )
                ps = pool.tile([P, 512], f32, tag="pp", name=f"pj_{which}{ot}{nch}")

                def mm(kstep):
                    nc.tensor.matmul(
                        ps,
                        lhsT=w_sb[:, kstep, ot * 128 : (ot + 1) * 128],
                        rhs=x_sb[:, kstep, nch * 512 : (nch + 1) * 512],
                        start=(kstep == 0),
                        stop=(kstep == KT - 1),
                    )

                def fin():
                    nc.vector.tensor_copy(
                        out=dst[:, ot, nch * 512 : (nch + 1) * 512], in_=ps
                    )

                return mm, fin

            def emit_qk_proj(ot, pool):
                for which in ("q", "k"):
                    for nch in range(2):
                        mm, fin = qk_group(which, ot, nch, pool)
                        for kstep in range(KT):
                            mm(kstep)
                        fin()

            def emit_v_proj():
                nc.vector.memset(vp_sb[:, :, :, D], 1.0)
                for mt in range(KT):
                    ps = ppp.tile([P, 512], f32, tag="pp", name=f"pv{mt}")
                    for kt in range(KT):
                        nc.tensor.matmul(
                            ps,
                            lhsT=vT_sb[:, kt, mt * 128 : (mt + 1) * 128],
                            rhs=wv_sb[:, kt, :],
                            start=(kt == 0),
                            stop=(kt == KT - 1),
                        )
                    nc.vector.tensor_copy(
                        out=vp_sb[:, mt, :, 0:D],
                        in_=ps.rearrange("p (h d) -> p h d", h=NH),
                    )

            # ---- attention pipeline ----
            # cycle c: scores+exp of head c interleave with ctx matmuls of
            # head c-1; the head c-2 tail (normalizer bcast, ctxn, attn
            # normalize + store) and the Q/K projections for upcoming head
            # pairs are woven in at fixed kt positions so the PE always has
            # work while the DVE reciprocal chain runs.
            expTs = {}
            ctxs = {}
            recfs = {}
            rbcs = {}

            def part1(t):
                # after ctx(t) is complete: reciprocal of the sums row
                ctx = ctxs[t]
                sums = small.tile([1, S], f32, tag="sums")
                nc.vector.tensor_copy(out=sums, in_=ctx[D : D + 1, :])
                recf = small.tile([1, S], f32, tag="recf")
                nc.vector.reciprocal_approx_fast(out=recf, in_=sums)
                recfs[t] = recf

            def part2(t):
                # broadcast 1/s via K=1 PE matmul, then ctxn (out-proj gate)
                ctx = ctxs.pop(t)
                recf = recfs.pop(t)
                rbc = small.tile([P, S], bf16, tag="rbc", name=f"rbc{t}")
                for nch in range(2):
                    bc = bcp.tile([P, 512], f32, tag="bc", name=f"bc{t}{nch}")
                    nc.tensor.matmul(
                        bc,
                        lhsT=ones_sb,
                        rhs=recf[:, nch * 512 : (nch + 1) * 512],
                        start=True,
                        stop=True,
                    )
                    nc.vector.tensor_copy(
                        out=rbc[:, nch * 512 : (nch + 1) * 512], in_=bc
                    )
                base = 64 * (t % 2)
                nc.vector.tensor_mul(
                    out=ctxn_sb[base : base + 64, t // 2, :],
                    in0=ctx[0:D, :],
                    in1=rbc[0:D, :],
                )
                rbcs[t] = rbc

            def part3(t):
                # attn normalization (one 3D op) + store
                expT = expTs.pop(t)
                rbc = rbcs.pop(t)
                nc.vector.tensor_mul(
                    out=expT,
                    in0=expT,
                    in1=rbc.unsqueeze(1).to_broadcast((P, KT, S)),
                )
                nc.sync.dma_start(out=attnT[t], in_=expT)

            def cycle(c, proj_ot=None):
                if c < NH:
                    expTs[c] = exps.tile([P, KT, S], bf16, tag="expT", name=f"expT{c}")
                # projections for head pair proj_ot run as two waves of two
                # concurrent [128, 512] groups, 2 matmuls per kt step
                pgroups = []
                if proj_ot is not None:
                    pgroups = [
                        qk_group("q", proj_ot, 0, ppp),
                        qk_group("q", proj_ot, 1, ppp),
                        qk_group("k", proj_ot, 0, ppp),
                        qk_group("k", proj_ot, 1, ppp),
                    ]
                for kt in range(KT):
                    if c < NH:
                        hp = 64 * (c % 2)
                        ot = c // 2
                        sc = scp.tile([P, S], f32, tag="sc", name=f"sc{c}{kt}")
                        for nch in range(2):
                            nc.tensor.matmul(
                                sc[:, nch * 512 : (nch + 1) * 512],
                                lhsT=kp_sb[
                                    hp : hp + 64, ot, kt * 128 : (kt + 1) * 128
                                ],
                                rhs=qp_sb[hp : hp + 64, ot, nch * 512 : (nch + 1) * 512],
                                start=True,
                                stop=True,
                            )
                        # exp(scores/8); bf16 out feeds both the ctx matmul
                        # and, after normalization, the attn output
                        nc.scalar.activation(
                            out=expTs[c][:, kt, :], in_=sc[:], func=Exp, scale=1.0 / 8.0
                        )
                    if c >= 1:
                        t = c - 1
                        if kt == 0:
                            ctxs[t] = ctxp.tile([P, S], f32, tag="ctx", name=f"ctx{t}")
                        for nch in range(2):
                            nc.tensor.matmul(
                                ctxs[t][0 : D + 1, nch * 512 : (nch + 1) * 512],
                                lhsT=vp_sb[:, kt, t, :],
                                rhs=expTs[t][:, kt, nch * 512 : (nch + 1) * 512],
                                start=(kt == 0),
                                stop=(kt == KT - 1),
                            )
                    if pgroups:
                        wave = pgroups[:2] if kt < 4 else pgroups[2:]
                        for g, (mm, _fin) in enumerate(wave):
                            mm(2 * (kt % 4) + 0)
                            mm(2 * (kt % 4) + 1)
                        if kt == 3 or kt == KT - 1:
                            for _mm, fin in (pgroups[:2] if kt == 3 else pgroups[2:]):
                                fin()
                    # head c-2's tail is interleaved at fixed positions so
                    # the PE keeps streaming while the DVE chain runs
                    if c >= 2 and kt == 1:
                        part2(c - 2)
                    if c >= 2 and kt == 3:
                        part3(c - 2)
                if c >= 1:
                    part1(c - 1)

            emit_qk_proj(0, ppp)
            cycle(0)                 # scores/exp head 0
            emit_v_proj()
            cycle(1, proj_ot=1)      # + ctx head 0
            for c in range(2, NH):
                cycle(c, proj_ot=(c // 2 + 1 if c % 2 == 1 and c // 2 + 1 < OT else None))
            cycle(NH)                # ctx head 7
            part2(NH - 1)

            # ---- output projection: outT = woT.T @ ctxn ----
            for mt in range(KT):
                for nch in range(2):
                    pslice = scp.tile([P, 512], f32, tag="sc", name=f"po{mt}{nch}")
                    for ct in range(OT):
                        nc.tensor.matmul(
                            pslice,
                            lhsT=wo_sb[:, ct, mt * 128 : (mt + 1) * 128],
                            rhs=ctxn_sb[:, ct, nch * 512 : (nch + 1) * 512],
                            start=(ct == 0),
                            stop=(ct == OT - 1),
                        )
                    ob = outsb.tile([P, 512], f32, tag="ob")
                    nc.scalar.copy(out=ob, in_=pslice)
                    nc.sync.dma_start(
                        out=outT[:, mt, nch * 512 : (nch + 1) * 512], in_=ob
                    )
                if mt == 0:
                    part3(NH - 1)

    nc.compile()
    return nc


def _get_nc():
    if "nc" not in _CACHE:
        _CACHE["nc"] = _build_module()
    return _CACHE["nc"]


def _make_in_maps(query, key, value, Wq, Wk, Wv, Wo):
    query, key, value, Wq, Wk, Wv, Wo = (
        np.asarray(x, dtype=np.float32) for x in (query, key, value, Wq, Wk, Wv, Wo)
    )
    in_maps = []
    for c in range(NCORES):
        b, half = divmod(c, 2)
        hs = slice(half * DH, (half + 1) * DH)
        def pmajor(arr2d):
            # [(t p), c] -> [(p t), c] so each SBUF partition's data is one
            # contiguous DRAM run
            r, cdim = arr2d.shape
            t = r // P
            return (
                arr2d.reshape(t, P, cdim).swapaxes(0, 1).reshape(r, cdim)
            )

        in_maps.append(
            {
                "qT": pmajor(query[b].T.astype(BF16NP)),
                "kT": pmajor(key[b].T.astype(BF16NP)),
                "vT": pmajor(value[b].T.astype(BF16NP)),
                "wqT": pmajor(Wq[hs, :].T.astype(BF16NP)),
                "wkT": pmajor(Wk[hs, :].T.astype(BF16NP)),
                "wvT": pmajor(Wv[hs, :].T.astype(BF16NP)),
                "woT": pmajor(Wo[:, hs].T.astype(BF16NP)),
            }
        )
    return in_maps


def _assemble(results):
    attn = np.empty((B, H, S, S), np.float32)
    out = np.empty((B, S, DM), np.float32)
    for b in range(B):
        r0, r1 = results[2 * b], results[2 * b + 1]
        # attnT is [h, p, t, q] with k = t*128+p; reference wants [h, q, k]
        for half, r in ((0, r0), (1, r1)):
            a = np.asarray(r["attnT"])  # [8, 128, 8, 1024]
            a = a.transpose(0, 3, 2, 1).reshape(NH, S, S)  # [h, q, (t p)=k]
            attn[b, half * NH : (half + 1) * NH] = a.astype(np.float32)
        # outT is [p, mt, q] with dm = mt*128+p
        o = np.asarray(r0["outT"]) + np.asarray(r1["outT"])
        out[b] = o.transpose(2, 1, 0).reshape(S, DM)
    return out, attn


def run(trace=False, **inputs):
    from concourse import bass_utils

    nc = _get_nc()
    in_maps = _make_in_maps(**inputs)
    res = bass_utils.run_bass_kernel_spmd(
        nc, in_maps, core_ids=list(range(NCORES)), trace=trace
    )
    _CACHE["last_result"] = res
    out, attn = _assemble(res.results)
    return out, attn


def kernel(query, key, value, Wq, Wk, Wv, Wo):
    return run(
        query=query, key=key, value=value, Wq=Wq, Wk=Wk, Wv=Wv, Wo=Wo
    )
